# revision 1
# baseline (speedup 1.0000x reference)
"""Bass/Trainium2 kernel for nn_DecoderAttn: batch-1 attention decoder step.

Sharding over 8 NeuronCores (tensor-parallel):
  - W_attn row-split (L dim): each core computes 512 attn logits -> AllGather
  - encoder_outputs col-split (H dim): each core computes 512 of attn_applied -> AllGather
  - W_comb row-split: 512 of lstm_in -> AllGather
  - W_ih/W_hh row-split by hidden slice (512 rows of each gate): LSTM math on
    a 512-slice of (c, h) -> AllGather h_new
  - W_out row-split (vocab dim, bf16): 16000 logits/core; log_softmax via an
    AllGather of per-core (max, sumexp) stats.

All matvecs run x-stationary on the tensor engine: lhsT = vector chunk
(128,1), rhs = weight tile (128, N<=512) streamed from HBM. Weights are
host-transposed/permuted into the exact device tile layout so every DMA is
large and contiguous. Chain weights stay fp32 (accuracy: LSTM gates saturate
at |g|~1000, bf16 there flips gates); W_out is bf16 (measured final absmax
err ~5e-3 on outputs of magnitude ~12).
"""

import sys

if '/opt/trn_rl_repo' not in sys.path:
    sys.path.insert(0, '/opt/trn_rl_repo')

import numpy as np

import concourse.bass as bass
import concourse.bacc as bacc
import concourse.tile as tile
import concourse.mybir as mybir
from concourse.bass_utils import run_bass_kernel_spmd

F32 = mybir.dt.float32
F16 = mybir.dt.float16

H = 4096
L = 4096
V = 128000
NC = 8
SH = H // NC        # 512 hidden slice
SL = L // NC        # 512 logit slice
SV = V // NC        # 16000 vocab slice

# stage-E vocab windows: chunks of <=512, <=4 chunks (psum banks) per window
def _windows():
    wins = []
    v = 0
    while v < SV:
        wlen = min(2000, SV - v)
        chunks = []
        c = 0
        while c < wlen:
            n = min(512, wlen - c)
            chunks.append((c, n))
            c += n
        wins.append((v, wlen, chunks))
        v += wlen
    return wins

WINDOWS = _windows()   # 8 windows of 2000: chunks 512,512,512,464
N_I = H // 128         # 32 contraction chunks for K=4096
N_I2 = 2 * H // 128    # 64 for K=8192

_compiled = {}


def _build():
    nc = bacc.Bacc("TRN2", target_bir_lowering=False, debug=False, num_devices=NC)

    # ---- kernel I/O (per-core shards, same names across cores) ----
    d_h0 = nc.dram_tensor("h0f", [H], F16, kind="ExternalInput")
    d_x0 = nc.dram_tensor("x0f", [H], F16, kind="ExternalInput")
    d_c0 = nc.dram_tensor("c0s", [SH], F32, kind="ExternalInput")
    d_ba = nc.dram_tensor("ba", [SL], F32, kind="ExternalInput")
    d_bc = nc.dram_tensor("bc", [SH], F32, kind="ExternalInput")
    d_bg = nc.dram_tensor("bg", [4 * SH], F32, kind="ExternalInput")
    d_bo = nc.dram_tensor("bo", [SV], F32, kind="ExternalInput")
    d_wa = nc.dram_tensor("wa", [8, 128, 8 * SL], F16, kind="ExternalInput")
    d_e = nc.dram_tensor("e", [4, 128, 8 * SH], F16, kind="ExternalInput")
    d_wc = nc.dram_tensor("wc", [8, 128, 8 * SH], F16, kind="ExternalInput")
    d_whh = nc.dram_tensor("whh", [16, 128, 2 * 2048], F16, kind="ExternalInput")
    d_wih = nc.dram_tensor("wih", [16, 128, 2 * 2048], F16, kind="ExternalInput")
    d_wo = nc.dram_tensor("wo", [8, 16, 128, 4000], F16, kind="ExternalInput")
    d_out = nc.dram_tensor("out", [1, SV], F32, kind="ExternalOutput")

    rg = [list(range(NC))]

    with tile.TileContext(nc) as tc:
        with (
            tc.tile_pool(name="singles", bufs=1) as sg,
            tc.tile_pool(name="cw", bufs=4) as cw,       # chain weight stream
            tc.tile_pool(name="wop", bufs=8) as wop,     # W_out stream
            tc.tile_pool(name="small", bufs=1) as sm,    # small working tiles
            tc.tile_pool(name="psum", bufs=1, space="PSUM") as ps,
            tc.tile_pool(name="dram", bufs=1, space="DRAM") as dr,
        ):
            # ---------- rank alignment barrier ----------
            # absorbs SPMD dispatch skew so the first real AllGather doesn't
            bar_in = dr.tile([1, 8], F32, tag="bar_in")
            bar_out = dr.tile([NC, 8], F32, tag="bar_out")
            zt = sg.tile([1, 8], F32, tag="zt")
            nc.gpsimd.memset(zt[:], 0.0)
            nc.gpsimd.dma_start(bar_in[:], zt[:])
            nc.gpsimd.collective_compute(
                "AllGather", mybir.AluOpType.bypass,
                ins=[bar_in.opt()], outs=[bar_out.opt()], replica_groups=rg)

            # ---------- small loads ----------
            hx = sg.tile([128, 64], F16, tag="hx")       # [h; x], elem 64p+i
            nc.sync.dma_start(hx[0:64, :], d_h0[:].rearrange("(p i) -> p i", p=64))
            nc.sync.dma_start(hx[64:128, :], d_x0[:].rearrange("(p i) -> p i", p=64))
            ht = sg.tile([128, 32], F16, tag="ht")       # h, elem 32p+i
            nc.sync.dma_start(ht[:], d_h0[:].rearrange("(p i) -> p i", p=128))
            c0t = sg.tile([1, SH], F32, tag="c0t")
            nc.sync.dma_start(c0t[:], d_c0[:].rearrange("n -> () n"))
            bat = sg.tile([1, SL], F32, tag="bat")
            nc.sync.dma_start(bat[:], d_ba[:].rearrange("n -> () n"))
            bct = sg.tile([1, SH], F32, tag="bct")
            nc.sync.dma_start(bct[:], d_bc[:].rearrange("n -> () n"))
            bgt = sg.tile([1, 4 * SH], F32, tag="bgt")
            nc.sync.dma_start(bgt[:], d_bg[:].rearrange("n -> () n"))

            # ---------- stage A: attn logits = [h;x] @ W_attn^T ----------
            pa = ps.tile([1, SL], F32, tag="po", bufs=4)
            for blk in range(4):
                wt = cw.tile([128, 2, 8, SL], F16, tag="cw")
                nc.sync.dma_start(wt[:], d_wa[2 * blk:2 * blk + 2]
                                  .rearrange("b p (j n) -> p b j n", j=8))
                for j in range(16):
                    i = 16 * blk + j
                    nc.tensor.matmul(pa[:], hx[:, i:i + 1],
                                     wt[:, j // 8, j % 8, :],
                                     start=(i == 0), stop=(i == N_I2 - 1))
            logits_loc = sm.tile([1, SL], F16, tag="vloc", bufs=2)
            nc.vector.tensor_add(logits_loc[:], pa[:], bat[:])
            ag_a_in = dr.tile([1, SL], F16, tag="agai")
            ag_a_out = dr.tile([NC, SL], F16, tag="agao")
            nc.gpsimd.dma_start(ag_a_in[:], logits_loc[:])
            nc.gpsimd.collective_compute(
                "AllGather", mybir.AluOpType.bypass,
                ins=[ag_a_in.opt()], outs=[ag_a_out.opt()], replica_groups=rg)

            # ---------- stage B: attn_applied with folded log_softmax ----------
            # aa = log_softmax(l) @ E = l @ E - logZ * (1^T E); the logZ
            # reduction chain runs on DVE/ACT concurrently with the matmuls.
            aw = sg.tile([128, 32], F16, tag="aw")       # raw logits
            nc.gpsimd.dma_start(
                aw[:],
                ag_a_out[:].rearrange("r n -> (r n)").rearrange("(p i) -> p i", p=128))
            lfl = sm.tile([1, L], F16, tag="lfl")        # logits, free-major
            nc.gpsimd.dma_start(
                lfl[:], ag_a_out[:].rearrange("r n -> (r n)").rearrange("n -> () n"))
            mxb = sm.tile([1, 1], F32, tag="mxb")
            nc.vector.tensor_reduce(mxb[:], lfl[:], mybir.AxisListType.X,
                                    mybir.AluOpType.max)
            nmxb = sm.tile([1, 1], F32, tag="nmxb")
            nc.vector.tensor_scalar_mul(nmxb[:], mxb[:], -1.0)
            s1 = sm.tile([1, 1], F32, tag="s1")
            nc.scalar.activation(lfl[:], lfl[:], mybir.ActivationFunctionType.Exp,
                                 bias=nmxb[:], accum_out=s1[:])
            lnsb = sm.tile([1, 1], F32, tag="lnsb")
            nc.scalar.activation(lnsb[:], s1[:], mybir.ActivationFunctionType.Ln)
            lzb = sm.tile([1, 1], F32, tag="lzb")
            nc.vector.tensor_add(lzb[:], mxb[:], lnsb[:])
            nlz = sm.tile([1, 1], F32, tag="nlz")
            nc.vector.tensor_scalar_mul(nlz[:], lzb[:], -1.0)
            ones = sg.tile([128, 1], F16, tag="ones")
            nc.vector.memset(ones[:], 1.0)
            pb = ps.tile([1, SH], F32, tag="po", bufs=4)
            pcs = ps.tile([1, SH], F32, tag="po", bufs=4, name="pcs")
            e_tiles = []
            for blk in range(2):
                et2 = cw.tile([128, 2, 8, SH], F16, tag="cw", name=f"et2_{blk}")
                nc.sync.dma_start(et2[:], d_e[2 * blk:2 * blk + 2]
                                  .rearrange("b p (j n) -> p b j n", j=8))
                e_tiles.append(et2)
                # colsum = 1^T E needs no AG result: fills the AllGather stall
                for j in range(16):
                    i = 16 * blk + j
                    nc.tensor.matmul(pcs[:], ones[:], et2[:, j // 8, j % 8, :],
                                     start=(i == 0), stop=(i == N_I - 1))
            for blk in range(2):
                et2 = e_tiles[blk]
                for j in range(16):
                    i = 16 * blk + j
                    nc.tensor.matmul(pb[:], aw[:, i:i + 1],
                                     et2[:, j // 8, j % 8, :],
                                     start=(i == 0), stop=(i == N_I - 1))
            cs_sb = sm.tile([1, SH], F32, tag="cs_sb")
            nc.vector.tensor_copy(cs_sb[:], pcs[:])
            aa_loc = sm.tile([1, SH], F16, tag="vloc", bufs=2)
            nc.vector.scalar_tensor_tensor(aa_loc[:], cs_sb[:], nlz[:], pb[:],
                                           mybir.AluOpType.mult,
                                           mybir.AluOpType.add)
            ag_b_in = dr.tile([1, SH], F16, tag="agbi")
            ag_b_out = dr.tile([NC, SH], F16, tag="agbo")
            nc.gpsimd.dma_start(ag_b_in[:], aa_loc[:])
            nc.gpsimd.collective_compute(
                "AllGather", mybir.AluOpType.bypass,
                ins=[ag_b_in.opt()], outs=[ag_b_out.opt()], replica_groups=rg)

            # ---------- stage C: lstm_in = [x; aa] @ W_comb^T ----------
            xc = sg.tile([128, 64], F16, tag="xc")
            nc.sync.dma_start(xc[0:64, :], d_x0[:].rearrange("(p i) -> p i", p=64))
            nc.gpsimd.dma_start(
                xc[64:128, :],
                ag_b_out[:].rearrange("r n -> (r n)").rearrange("(p i) -> p i", p=64))
            pc = ps.tile([1, SH], F32, tag="po", bufs=4)
            for blk in range(4):
                wt = cw.tile([128, 2, 8, SH], F16, tag="cw")
                nc.sync.dma_start(wt[:], d_wc[2 * blk:2 * blk + 2]
                                  .rearrange("b p (j n) -> p b j n", j=8))
                for j in range(16):
                    i = 16 * blk + j
                    nc.tensor.matmul(pc[:], xc[:, i:i + 1],
                                     wt[:, j // 8, j % 8, :],
                                     start=(i == 0), stop=(i == N_I2 - 1))
            li_loc = sm.tile([1, SH], F16, tag="vloc", bufs=2)
            nc.vector.tensor_add(li_loc[:], pc[:], bct[:])
            ag_c_in = dr.tile([1, SH], F16, tag="agci")
            ag_c_out = dr.tile([NC, SH], F16, tag="agco")
            nc.gpsimd.dma_start(ag_c_in[:], li_loc[:])
            nc.gpsimd.collective_compute(
                "AllGather", mybir.AluOpType.bypass,
                ins=[ag_c_in.opt()], outs=[ag_c_out.opt()], replica_groups=rg)

            # ---------- stage D: gates + LSTM cell ----------
            li = sg.tile([128, 32], F16, tag="li")
            nc.gpsimd.dma_start(
                li[:],
                ag_c_out[:].rearrange("r n -> (r n)").rearrange("(p i) -> p i", p=128))
            pg = ps.tile([1, 4 * SH], F32, tag="pg")
            # h @ W_hh^T first (h is ready at t=0)
            for blk in range(16):
                wt = cw.tile([128, 2, 2048], F16, tag="cw")
                nc.sync.dma_start(wt[:], d_whh[blk].rearrange("p (j n) -> p j n", j=2))
                for j in range(2):
                    i = 2 * blk + j
                    for b in range(4):
                        nc.tensor.matmul(pg[:, 512 * b:512 * (b + 1)],
                                         ht[:, i:i + 1],
                                         wt[:, j, 512 * b:512 * (b + 1)],
                                         start=(i == 0), stop=False)
            # + lstm_in @ W_ih^T
            for blk in range(16):
                wt = cw.tile([128, 2, 2048], F16, tag="cw")
                nc.sync.dma_start(wt[:], d_wih[blk].rearrange("p (j n) -> p j n", j=2))
                for j in range(2):
                    i = 2 * blk + j
                    for b in range(4):
                        nc.tensor.matmul(pg[:, 512 * b:512 * (b + 1)],
                                         li[:, i:i + 1],
                                         wt[:, j, 512 * b:512 * (b + 1)],
                                         start=False, stop=(i == N_I - 1))
            gsb = sm.tile([1, 4 * SH], F32, tag="gsb")
            nc.vector.tensor_add(gsb[:], pg[:], bgt[:])
            s_i = sm.tile([1, SH], F32, tag="si")
            s_f = sm.tile([1, SH], F32, tag="sf")
            t_g = sm.tile([1, SH], F32, tag="tg")
            s_o = sm.tile([1, SH], F32, tag="so")
            Sg = mybir.ActivationFunctionType.Sigmoid
            Th = mybir.ActivationFunctionType.Tanh
            nc.scalar.activation(s_i[:], gsb[:, 0:SH], Sg)
            nc.scalar.activation(s_f[:], gsb[:, SH:2 * SH], Sg)
            nc.scalar.activation(s_o[:], gsb[:, 3 * SH:4 * SH], Sg)
            nc.scalar.activation(t_g[:], gsb[:, 2 * SH:3 * SH], Th)
            cf = sm.tile([1, SH], F32, tag="cf")
            nc.vector.tensor_mul(cf[:], s_f[:], c0t[:])
            ci = sm.tile([1, SH], F32, tag="ci")
            nc.vector.tensor_mul(ci[:], s_i[:], t_g[:])
            cn = sm.tile([1, SH], F32, tag="cn")
            nc.vector.tensor_add(cn[:], cf[:], ci[:])
            tc_n = sm.tile([1, SH], F32, tag="tcn")
            nc.scalar.activation(tc_n[:], cn[:], Th)
            hn_loc = sm.tile([1, SH], F16, tag="vloc", bufs=2)
            nc.vector.tensor_mul(hn_loc[:], s_o[:], tc_n[:])
            ag_h_in = dr.tile([1, SH], F16, tag="aghi")
            ag_h_out = dr.tile([NC, SH], F16, tag="agho")
            nc.gpsimd.dma_start(ag_h_in[:], hn_loc[:])
            nc.gpsimd.collective_compute(
                "AllGather", mybir.AluOpType.bypass,
                ins=[ag_h_in.opt()], outs=[ag_h_out.opt()], replica_groups=rg)

            # ---------- stage E: word = h_new @ W_out^T (bf16) ----------
            hn = sg.tile([128, 32], F16, tag="hn")
            nc.gpsimd.dma_start(
                hn[:],
                ag_h_out[:].rearrange("r n -> (r n)").rearrange("(p i) -> p i", p=128))

            word_dram = dr.tile([1, SV], F32, tag="wordd")
            n_chunks = sum(len(w[2]) for w in WINDOWS)
            mx_sb = sg.tile([1, n_chunks], F32, tag="mxsb")
            sm_sb = sg.tile([1, n_chunks], F32, tag="smsb")
            cidx = 0
            for (vw, wlen, chunks) in WINDOWS:
                po = [ps.tile([1, 512], F32, tag="po", bufs=4, name=f"po_{vw}_{c}")
                      for c in range(len(chunks))]
                w_idx = vw // 2000
                for ib in range(16):
                    wt = wop.tile([128, 2, 2000], F16, tag="wo")
                    nc.scalar.dma_start(wt[:],
                                        d_wo[w_idx, ib].rearrange("p (j n) -> p j n", j=2))
                    for j in range(2):
                        i = 2 * ib + j
                        for c, (coff, n) in enumerate(chunks):
                            nc.tensor.matmul(po[c][:, 0:n], hn[:, i:i + 1],
                                             wt[:, j, coff:coff + n],
                                             start=(i == 0), stop=(i == N_I - 1))
                for c, (coff, n) in enumerate(chunks):
                    vabs = vw + coff
                    bo_c = sm.tile([1, 512], F32, tag="boc", bufs=4)
                    nc.sync.dma_start(bo_c[:, 0:n],
                                        d_bo[vabs:vabs + n].rearrange("n -> () n"))
                    stg = sm.tile([1, 512], F32, tag="stg", bufs=4)
                    nc.vector.tensor_add(stg[:, 0:n], po[c][:, 0:n], bo_c[:, 0:n])
                    nc.gpsimd.dma_start(word_dram[:, vabs:vabs + n], stg[:, 0:n])
                    nc.vector.tensor_reduce(mx_sb[:, cidx:cidx + 1], stg[:, 0:n],
                                            mybir.AxisListType.X, mybir.AluOpType.max)
                    nmx_c = sm.tile([1, 1], F32, tag="nmxc", bufs=2)
                    nc.vector.tensor_scalar_mul(nmx_c[:], mx_sb[:, cidx:cidx + 1], -1.0)
                    esc = sm.tile([1, 512], F32, tag="esc", bufs=2)
                    nc.scalar.activation(esc[:, 0:n], stg[:, 0:n],
                                         mybir.ActivationFunctionType.Exp,
                                         bias=nmx_c[:],
                                         accum_out=sm_sb[:, cidx:cidx + 1])
                    cidx += 1

            # ---------- local stats -> global logsumexp ----------
            m_loc = sm.tile([1, 1], F32, tag="mloc")
            nc.vector.tensor_reduce(m_loc[:], mx_sb[:], mybir.AxisListType.X,
                                    mybir.AluOpType.max)
            dm = sm.tile([1, n_chunks], F32, tag="dm")
            nc.vector.tensor_scalar(dm[:], mx_sb[:], m_loc[:], None,
                                    mybir.AluOpType.subtract)
            edm = sm.tile([1, n_chunks], F32, tag="edm")
            nc.scalar.activation(edm[:], dm[:], mybir.ActivationFunctionType.Exp)
            sadj = sm.tile([1, n_chunks], F32, tag="sadj")
            nc.vector.tensor_mul(sadj[:], sm_sb[:], edm[:])
            s_loc = sm.tile([1, 1], F32, tag="sloc")
            nc.vector.tensor_reduce(s_loc[:], sadj[:], mybir.AxisListType.X,
                                    mybir.AluOpType.add)
            pack = sm.tile([1, 8], F32, tag="pack")
            nc.vector.tensor_copy(pack[:, 0:1], m_loc[:])
            nc.vector.tensor_copy(pack[:, 1:2], s_loc[:])
            ag_s_in = dr.tile([1, 8], F32, tag="agsi")
            ag_s_out = dr.tile([NC, 8], F32, tag="agso")
            nc.gpsimd.dma_start(ag_s_in[:], pack[:])
            nc.gpsimd.collective_compute(
                "AllGather", mybir.AluOpType.bypass,
                ins=[ag_s_in.opt()], outs=[ag_s_out.opt()], replica_groups=rg)
            stat = sm.tile([1, NC, 8], F32, tag="stat")
            nc.gpsimd.dma_start(stat[:], ag_s_out[:].rearrange("r n -> () r n"))
            m_all = stat[:, :, 0]
            s_all = stat[:, :, 1]
            gm = sm.tile([1, 1], F32, tag="gm")
            nc.vector.tensor_reduce(gm[:], m_all, mybir.AxisListType.X,
                                    mybir.AluOpType.max)
            dg = sm.tile([1, NC], F32, tag="dg")
            nc.vector.tensor_scalar(dg[:], m_all, gm[:], None,
                                    mybir.AluOpType.subtract)
            edg = sm.tile([1, NC], F32, tag="edg")
            nc.scalar.activation(edg[:], dg[:], mybir.ActivationFunctionType.Exp)
            sg2 = sm.tile([1, NC], F32, tag="sg2")
            nc.vector.tensor_mul(sg2[:], s_all, edg[:])
            ssum = sm.tile([1, 1], F32, tag="ssum")
            nc.vector.tensor_reduce(ssum[:], sg2[:], mybir.AxisListType.X,
                                    mybir.AluOpType.add)
            lns2 = sm.tile([1, 1], F32, tag="lns2")
            nc.scalar.activation(lns2[:], ssum[:], mybir.ActivationFunctionType.Ln)
            lse = sm.tile([1, 1], F32, tag="lse")
            nc.vector.tensor_add(lse[:], gm[:], lns2[:])

            # ---------- out = word - lse ----------
            QT = SV // 8
            for hf in range(8):
                wh = sm.tile([1, QT], F32, tag="wh")
                nc.gpsimd.dma_start(wh[:], word_dram[:, hf * QT:(hf + 1) * QT])
                nc.vector.tensor_scalar(wh[:], wh[:], lse[:], None,
                                        mybir.AluOpType.subtract)
                nc.gpsimd.dma_start(d_out[:, hf * QT:(hf + 1) * QT], wh[:])

    nc.compile()
    return nc


def _get_nc():
    if 'nc' not in _compiled:
        _compiled['nc'] = _build()
    return _compiled['nc']


def _shard_inputs(encoder_outputs, h0, c0, x0, W_attn, b_attn, W_comb, b_comb,
                  W_ih, b_ih, W_hh, b_hh, W_out, b_out):
    f = lambda a: np.ascontiguousarray(np.asarray(a), dtype=np.float32)
    E = f(encoder_outputs); W_attn = f(W_attn); W_comb = f(W_comb)
    W_ih = f(W_ih); W_hh = f(W_hh); W_out = f(W_out)
    h0f = f(h0).reshape(H).astype(np.float16)
    x0f = f(x0).reshape(H).astype(np.float16)
    c0f = f(c0).reshape(H)
    b_attn = f(b_attn); b_comb = f(b_comb); b_out = f(b_out)
    bg_full = f(b_ih) + f(b_hh)

    # E chunks: [blk, p, j, n] = E[32p + 8blk + j, h0+n]
    E_r = E.reshape(128, 32, H)

    in_maps = []
    for k in range(NC):
        l0, hh0, v0 = k * SL, k * SH, k * SV
        wa = W_attn[l0:l0 + SL].T.reshape(128, 8, 8, SL) \
            .transpose(1, 0, 2, 3).reshape(8, 128, 8 * SL)
        e = E_r[:, :, hh0:hh0 + SH].reshape(128, 4, 8, SH) \
            .transpose(1, 0, 2, 3).reshape(4, 128, 8 * SH)
        wc = W_comb[hh0:hh0 + SH].T.reshape(128, 8, 8, SH) \
            .transpose(1, 0, 2, 3).reshape(8, 128, 8 * SH)
        rows = np.concatenate([np.arange(g * H + hh0, g * H + hh0 + SH)
                               for g in range(4)])
        wih = W_ih[rows].T.reshape(128, 16, 2, 2048) \
            .transpose(1, 0, 2, 3).reshape(16, 128, 4096)
        whh = W_hh[rows].T.reshape(128, 16, 2, 2048) \
            .transpose(1, 0, 2, 3).reshape(16, 128, 4096)
        G = W_out[v0:v0 + SV].T.reshape(128, 32, SV).transpose(1, 0, 2)
        wo = np.ascontiguousarray(
            G.reshape(16, 2, 128, 8, 2000).transpose(3, 0, 2, 1, 4)
        ).astype(np.float16).reshape(8, 16, 128, 4000)
        in_maps.append({
            "h0f": h0f, "x0f": x0f, "c0s": np.ascontiguousarray(c0f[hh0:hh0 + SH]),
            "ba": np.ascontiguousarray(b_attn[l0:l0 + SL]),
            "bc": np.ascontiguousarray(b_comb[hh0:hh0 + SH]),
            "bg": np.ascontiguousarray(bg_full[rows]),
            "bo": np.ascontiguousarray(b_out[v0:v0 + SV]),
            "wa": np.ascontiguousarray(wa, np.float16),
            "e": np.ascontiguousarray(e, np.float16),
            "wc": np.ascontiguousarray(wc, np.float16),
            "wih": np.ascontiguousarray(wih, np.float16),
            "whh": np.ascontiguousarray(whh, np.float16),
            "wo": wo,
        })
    return in_maps


def _run(in_maps, trace=False):
    nc = _get_nc()
    return run_bass_kernel_spmd(nc, in_maps, list(range(NC)), trace=trace)


def kernel(**inputs):
    in_maps = _shard_inputs(**inputs)
    res = _run(in_maps)
    return np.concatenate([res.results[k]["out"] for k in range(NC)], axis=1)


def run_traced(**inputs):
    """test-only helper: returns (output, BassKernelResults with profiling)."""
    in_maps = _shard_inputs(**inputs)
    res = _run(in_maps, trace=True)
    out = np.concatenate([res.results[k]["out"] for k in range(NC)], axis=1)
    return out, res



# revision 5
# speedup vs baseline: 1.3531x; 1.3531x over previous
"""Bass/Trainium2 kernel for nn_DecoderAttn: batch-1 attention decoder step.

Sharding over 8 NeuronCores (tensor-parallel):
  - W_attn row-split (L dim): each core computes 512 attn logits -> AllGather
  - encoder_outputs col-split (H dim): each core computes 512 of attn_applied -> AllGather
  - W_comb row-split: 512 of lstm_in -> AllGather
  - W_ih/W_hh row-split by hidden slice (512 rows of each gate): LSTM math on
    a 512-slice of (c, h) -> AllGather h_new
  - W_out row-split (vocab dim): 16000 logits/core; log_softmax via an
    AllGather of per-core (max, sumexp) stats.

Precision plan (validated vs fp32 reference host-side, rel err ~6e-3):
  - W_out: fp8 e4m3 scaled x128 + DoubleRow matmul (2 k-subtiles of 128 per
    instruction) -> half the HBM bytes at full column rate; psum carries
    128*logits, rescaled in the fused bias-add.
  - W_hh: fp8 e4m3 natural scale + DoubleRow; its term is ~0.1% of the gate
    magnitude so fp8 noise there is negligible; accumulates into the same
    psum group as the fp16 W_ih matmuls.
  - Everything else fp16 weights / fp32 psum (fp8 anywhere else flips LSTM
    gates: measured 3-5e-2 rel err host-side).

DMA queues: chain weights (W_attn/E/W_comb/W_hh/W_ih) stream on the sync
ring; W_out streams on the scalar ring (all 128 tile DMAs issued up-front,
paced by the wop pool ring's WAR semaphores); small loads ride vector;
gpsimd handles collective staging. Stage-E logits stay in SBUF (fp16) and
the final log_softmax subtraction happens in-SBUF (vector+scalar halves),
removing the baseline's serialized DRAM round-trip tail.
"""

import sys

if '/opt/trn_rl_repo' not in sys.path:
    sys.path.insert(0, '/opt/trn_rl_repo')

import numpy as np
import ml_dtypes

import concourse.bass as bass
import concourse.bacc as bacc
import concourse.tile as tile
import concourse.mybir as mybir
from concourse.bass_utils import run_bass_kernel_spmd

F32 = mybir.dt.float32
F16 = mybir.dt.float16
F8 = mybir.dt.float8e4
DR = mybir.MatmulPerfMode.DoubleRow
E4 = ml_dtypes.float8_e4m3

H = 4096
L = 4096
V = 128000
NC = 8
SH = H // NC        # 512 hidden slice
SL = L // NC        # 512 logit slice
SV = V // NC        # 16000 vocab slice
ND = H // 256       # 16 DoubleRow k-pairs
NG = 8              # stage-E groups (4 chunks of 500 each)
NCH = 4             # chunks per group
CW = 500            # chunk width (NG*NCH*CW == SV)
WSC = 128.0         # W_out fp8 scale

N_I = H // 128      # 32 contraction chunks for K=4096
N_I2 = 2 * H // 128  # 64 for K=8192

_compiled = {}


def _build():
    nc = bacc.Bacc("TRN2", target_bir_lowering=False, debug=False, num_devices=NC)

    # ---- kernel I/O (per-core shards, same names across cores) ----
    d_h0 = nc.dram_tensor("h0f", [H], F16, kind="ExternalInput")
    d_x0 = nc.dram_tensor("x0f", [H], F16, kind="ExternalInput")
    d_h08 = nc.dram_tensor("h08", [128, 2, ND], F8, kind="ExternalInput")
    d_c0 = nc.dram_tensor("c0s", [SH], F32, kind="ExternalInput")
    d_ba = nc.dram_tensor("ba", [SL], F32, kind="ExternalInput")
    d_bc = nc.dram_tensor("bc", [SH], F32, kind="ExternalInput")
    d_bg = nc.dram_tensor("bg", [4 * SH], F32, kind="ExternalInput")
    d_bo = nc.dram_tensor("bo", [SV], F32, kind="ExternalInput")
    d_wa = nc.dram_tensor("wa", [8, 128, 8 * SL], F16, kind="ExternalInput")
    d_e = nc.dram_tensor("e", [4, 128, 8 * SH], F16, kind="ExternalInput")
    d_wc = nc.dram_tensor("wc", [8, 128, 8 * SH], F16, kind="ExternalInput")
    d_whh = nc.dram_tensor("whh", [ND, 128, 2, 4 * SH], F8, kind="ExternalInput")
    d_wih = nc.dram_tensor("wih", [16, 128, 2 * 2048], F16, kind="ExternalInput")
    d_wo = nc.dram_tensor("wo", [NG, ND, 128, 2, NCH * CW], F8, kind="ExternalInput")
    d_out = nc.dram_tensor("out", [1, SV], F32, kind="ExternalOutput")

    rg = [list(range(NC))]

    with tile.TileContext(nc) as tc:
        with (
            tc.tile_pool(name="singles", bufs=1) as sg,
            tc.tile_pool(name="cw", bufs=5) as cw,        # chain weight stream
            tc.tile_pool(name="wop", bufs=14) as wop,     # W_out stream
            tc.tile_pool(name="small", bufs=1) as sm,     # small working tiles
            tc.tile_pool(name="psum", bufs=1, space="PSUM") as ps,
            tc.tile_pool(name="dram", bufs=1, space="DRAM") as dr,
        ):
            # ---------- rank alignment barrier ----------
            bar_in = dr.tile([1, 8], F32, tag="bar_in")
            bar_out = dr.tile([NC, 8], F32, tag="bar_out")
            zt = sg.tile([1, 8], F32, tag="zt")
            nc.gpsimd.memset(zt[:], 0.0)
            nc.gpsimd.dma_start(bar_in[:], zt[:])
            nc.gpsimd.collective_compute(
                "AllGather", mybir.AluOpType.bypass,
                ins=[bar_in.opt()], outs=[bar_out.opt()], replica_groups=rg)

            # ---------- small loads (sync ring, ahead of the chain stream) --
            hx = sg.tile([128, 64], F16, tag="hx")       # [h; x], elem 64p+i
            nc.sync.dma_start(hx[0:64, :], d_h0[:].rearrange("(p i) -> p i", p=64))
            nc.sync.dma_start(hx[64:128, :], d_x0[:].rearrange("(p i) -> p i", p=64))
            h08 = sg.tile([128, 2, ND], F8, tag="h08")   # h for W_hh DoubleRow
            nc.sync.dma_start(h08[:], d_h08[:])
            xc = sg.tile([128, 64], F16, tag="xc")       # stage-C lhsT (x half)
            nc.sync.dma_start(xc[0:64, :], d_x0[:].rearrange("(p i) -> p i", p=64))
            c0t = sg.tile([1, SH], F32, tag="c0t")
            nc.sync.dma_start(c0t[:], d_c0[:].rearrange("n -> () n"))
            bat = sg.tile([1, SL], F32, tag="bat")
            nc.sync.dma_start(bat[:], d_ba[:].rearrange("n -> () n"))
            bct = sg.tile([1, SH], F32, tag="bct")
            nc.sync.dma_start(bct[:], d_bc[:].rearrange("n -> () n"))
            bgt = sg.tile([1, 4 * SH], F32, tag="bgt")
            nc.sync.dma_start(bgt[:], d_bg[:].rearrange("n -> () n"))

            # ---------- chain weight stream: issue ALL dmas up-front ----
            # (sync ring; paced by the cw pool's 6-slot WAR ring)
            wa_t, e_t, wc_t, whh_t, wih_t = [], [], [], [], []
            for b in range(8):
                t = cw.tile([128, 8, SL], F16, tag="cw", name=f"wa{b}")
                nc.sync.dma_start(t[:], d_wa[b].rearrange("p (j n) -> p j n", j=8))
                wa_t.append(t)
            for b in range(4):
                t = cw.tile([128, 8, SH], F16, tag="cw", name=f"e{b}")
                nc.sync.dma_start(t[:], d_e[b].rearrange("p (j n) -> p j n", j=8))
                e_t.append(t)
            for b in range(8):
                t = cw.tile([128, 8, SH], F16, tag="cw", name=f"wc{b}")
                nc.sync.dma_start(t[:], d_wc[b].rearrange("p (j n) -> p j n", j=8))
                wc_t.append(t)
            for b in range(ND):
                t = cw.tile([128, 2, 4 * SH], F8, tag="cw", name=f"whh{b}")
                nc.sync.dma_start(t[:], d_whh[b])
                whh_t.append(t)
            for b in range(16):
                t = cw.tile([128, 2, 2048], F16, tag="cw", name=f"wih{b}")
                nc.sync.dma_start(t[:], d_wih[b].rearrange("p (j n) -> p j n", j=2))
                wih_t.append(t)

            # ---------- W_out stream: issue ALL dmas up-front (scalar ring) --
            wo_t = []
            for g in range(NG):
                for d in range(ND):
                    t = wop.tile([128, 2, NCH * CW], F8, tag="wo",
                                 name=f"wo{g}_{d}")
                    nc.scalar.dma_start(t[:], d_wo[g, d])
                    wo_t.append(t)

            # ---------- stage A: attn logits = [h;x] @ W_attn^T ----------
            pa = ps.tile([1, SL], F32, tag="po", bufs=4)
            for b in range(8):
                for j in range(8):
                    i = 8 * b + j
                    nc.tensor.matmul(pa[:], hx[:, i:i + 1], wa_t[b][:, j, :],
                                     start=(i == 0), stop=(i == N_I2 - 1))
            logits_loc = sm.tile([1, SL], F16, tag="vloc", bufs=2)
            nc.vector.tensor_add(logits_loc[:], pa[:], bat[:])
            ag_a_in = dr.tile([1, SL], F16, tag="agai")
            ag_a_out = dr.tile([NC, SL], F16, tag="agao")
            nc.gpsimd.dma_start(ag_a_in[:], logits_loc[:])
            nc.gpsimd.collective_compute(
                "AllGather", mybir.AluOpType.bypass,
                ins=[ag_a_in.opt()], outs=[ag_a_out.opt()], replica_groups=rg)

            # ---------- stage B: attn_applied with folded log_softmax ----------
            # aa = log_softmax(l) @ E = l @ E - logZ * (1^T E)
            aw = sg.tile([128, 32], F16, tag="aw")       # raw logits
            nc.gpsimd.dma_start(
                aw[:],
                ag_a_out[:].rearrange("r n -> (r n)").rearrange("(p i) -> p i", p=128))
            lfl = sm.tile([1, L], F16, tag="lfl")        # logits, free-major
            nc.gpsimd.dma_start(
                lfl[:], ag_a_out[:].rearrange("r n -> (r n)").rearrange("n -> () n"))
            mxb = sm.tile([1, 1], F32, tag="mxb")
            nc.vector.tensor_reduce(mxb[:], lfl[:], mybir.AxisListType.X,
                                    mybir.AluOpType.max)
            nmxb = sm.tile([1, 1], F32, tag="nmxb")
            nc.vector.tensor_scalar_mul(nmxb[:], mxb[:], -1.0)
            s1 = sm.tile([1, 1], F32, tag="s1")
            nc.scalar.activation(lfl[:], lfl[:], mybir.ActivationFunctionType.Exp,
                                 bias=nmxb[:], accum_out=s1[:])
            lnsb = sm.tile([1, 1], F32, tag="lnsb")
            nc.scalar.activation(lnsb[:], s1[:], mybir.ActivationFunctionType.Ln)
            lzb = sm.tile([1, 1], F32, tag="lzb")
            nc.vector.tensor_add(lzb[:], mxb[:], lnsb[:])
            nlz = sm.tile([1, 1], F32, tag="nlz")
            nc.vector.tensor_scalar_mul(nlz[:], lzb[:], -1.0)
            ones = sg.tile([128, 1], F16, tag="ones")
            nc.vector.memset(ones[:], 1.0)
            pb = ps.tile([1, SH], F32, tag="po", bufs=4)
            pcs = ps.tile([1, SH], F32, tag="po", bufs=4, name="pcs")
            # colsum = 1^T E needs no AG result: fills the AllGather stall
            for b in range(4):
                for j in range(8):
                    i = 8 * b + j
                    nc.tensor.matmul(pcs[:], ones[:], e_t[b][:, j, :],
                                     start=(i == 0), stop=(i == N_I - 1))
            for b in range(4):
                for j in range(8):
                    i = 8 * b + j
                    nc.tensor.matmul(pb[:], aw[:, i:i + 1], e_t[b][:, j, :],
                                     start=(i == 0), stop=(i == N_I - 1))
            cs_sb = sm.tile([1, SH], F32, tag="cs_sb")
            nc.vector.tensor_copy(cs_sb[:], pcs[:])
            aa_loc = sm.tile([1, SH], F16, tag="vloc", bufs=2)
            nc.vector.scalar_tensor_tensor(aa_loc[:], cs_sb[:], nlz[:], pb[:],
                                           mybir.AluOpType.mult,
                                           mybir.AluOpType.add)
            ag_b_in = dr.tile([1, SH], F16, tag="agbi")
            ag_b_out = dr.tile([NC, SH], F16, tag="agbo")
            nc.gpsimd.dma_start(ag_b_in[:], aa_loc[:])
            nc.gpsimd.collective_compute(
                "AllGather", mybir.AluOpType.bypass,
                ins=[ag_b_in.opt()], outs=[ag_b_out.opt()], replica_groups=rg)

            # ---------- stage C: lstm_in = [x; aa] @ W_comb^T ----------
            nc.gpsimd.dma_start(
                xc[64:128, :],
                ag_b_out[:].rearrange("r n -> (r n)").rearrange("(p i) -> p i", p=64))
            pc = ps.tile([1, SH], F32, tag="po", bufs=4)
            for b in range(8):
                for j in range(8):
                    i = 8 * b + j
                    nc.tensor.matmul(pc[:], xc[:, i:i + 1], wc_t[b][:, j, :],
                                     start=(i == 0), stop=(i == N_I2 - 1))
            li_loc = sm.tile([1, SH], F16, tag="vloc", bufs=2)
            nc.vector.tensor_add(li_loc[:], pc[:], bct[:])
            ag_c_in = dr.tile([1, SH], F16, tag="agci")
            ag_c_out = dr.tile([NC, SH], F16, tag="agco")
            nc.gpsimd.dma_start(ag_c_in[:], li_loc[:])
            nc.gpsimd.collective_compute(
                "AllGather", mybir.AluOpType.bypass,
                ins=[ag_c_in.opt()], outs=[ag_c_out.opt()], replica_groups=rg)

            # ---------- stage D: gates + LSTM cell ----------
            li = sg.tile([128, 32], F16, tag="li")
            nc.gpsimd.dma_start(
                li[:],
                ag_c_out[:].rearrange("r n -> (r n)").rearrange("(p i) -> p i", p=128))
            pg = ps.tile([1, 4 * SH], F32, tag="pg")
            # h @ W_hh^T first: fp8 DoubleRow, h is ready at t=0
            for d in range(ND):
                for b in range(4):
                    nc.tensor.matmul(pg[:, 512 * b:512 * (b + 1)],
                                     h08[:, :, d:d + 1],
                                     whh_t[d][:, :, 512 * b:512 * (b + 1)],
                                     start=(d == 0), stop=False, perf_mode=DR)
            # + lstm_in @ W_ih^T (fp16)
            for blk in range(16):
                for j in range(2):
                    i = 2 * blk + j
                    for b in range(4):
                        nc.tensor.matmul(pg[:, 512 * b:512 * (b + 1)],
                                         li[:, i:i + 1],
                                         wih_t[blk][:, j, 512 * b:512 * (b + 1)],
                                         start=False, stop=(i == N_I - 1))
            gsb = sm.tile([1, 4 * SH], F32, tag="gsb")
            nc.vector.tensor_add(gsb[:], pg[:], bgt[:])
            s_i = sm.tile([1, SH], F32, tag="si")
            s_f = sm.tile([1, SH], F32, tag="sf")
            t_g = sm.tile([1, SH], F32, tag="tg")
            s_o = sm.tile([1, SH], F32, tag="so")
            Sg = mybir.ActivationFunctionType.Sigmoid
            Th = mybir.ActivationFunctionType.Tanh
            nc.scalar.activation(s_i[:], gsb[:, 0:SH], Sg)
            nc.scalar.activation(s_f[:], gsb[:, SH:2 * SH], Sg)
            nc.scalar.activation(s_o[:], gsb[:, 3 * SH:4 * SH], Sg)
            nc.scalar.activation(t_g[:], gsb[:, 2 * SH:3 * SH], Th)
            cf = sm.tile([1, SH], F32, tag="cf")
            nc.vector.tensor_mul(cf[:], s_f[:], c0t[:])
            ci = sm.tile([1, SH], F32, tag="ci")
            nc.vector.tensor_mul(ci[:], s_i[:], t_g[:])
            cn = sm.tile([1, SH], F32, tag="cn")
            nc.vector.tensor_add(cn[:], cf[:], ci[:])
            tc_n = sm.tile([1, SH], F32, tag="tcn")
            nc.scalar.activation(tc_n[:], cn[:], Th)
            hn_loc = sm.tile([1, SH], F16, tag="vloc", bufs=2)
            nc.vector.tensor_mul(hn_loc[:], s_o[:], tc_n[:])
            ag_h_in = dr.tile([1, SH], F16, tag="aghi")
            ag_h_out = dr.tile([NC, SH], F16, tag="agho")
            nc.gpsimd.dma_start(ag_h_in[:], hn_loc[:])
            nc.gpsimd.collective_compute(
                "AllGather", mybir.AluOpType.bypass,
                ins=[ag_h_in.opt()], outs=[ag_h_out.opt()], replica_groups=rg)

            # ---------- stage E: word = h_new @ W_out^T (fp8 DoubleRow) ----
            hn16 = sg.tile([128, 2, ND], F16, tag="hn16")
            nc.gpsimd.dma_start(
                hn16[:],
                ag_h_out[:].rearrange("r n -> (r n)")
                .rearrange("(p s d) -> p s d", p=128, s=2))
            hn8 = sg.tile([128, 2, ND], F8, tag="hn8")
            nc.vector.tensor_copy(hn8[:], hn16[:])

            word_sb = sg.tile([1, SV], F16, tag="word")
            n_chunks = NG * NCH
            mx_sb = sg.tile([1, n_chunks], F32, tag="mxsb")
            sm_sb = sg.tile([1, n_chunks], F32, tag="smsb")
            for g in range(NG):
                po = [ps.tile([1, 512], F32, tag="po", bufs=4, name=f"po{g}_{c}")
                      for c in range(NCH)]
                for d in range(ND):
                    wt = wo_t[g * ND + d]
                    for c in range(NCH):
                        nc.tensor.matmul(po[c][:, 0:CW], hn8[:, :, d:d + 1],
                                         wt[:, :, CW * c:CW * (c + 1)],
                                         start=(d == 0), stop=(d == ND - 1),
                                         perf_mode=DR)
                for c in range(NCH):
                    cidx = NCH * g + c
                    vabs = cidx * CW
                    bo_c = sm.tile([1, 512], F32, tag="boc", bufs=4)
                    nc.gpsimd.dma_start(bo_c[:, 0:CW],
                                        d_bo[vabs:vabs + CW].rearrange("n -> () n"))
                    # word = psum/128 + b_out (fused rescale of the x128 fp8)
                    wslice = word_sb[:, vabs:vabs + CW]
                    nc.vector.scalar_tensor_tensor(wslice, po[c][:, 0:CW],
                                                   1.0 / WSC, bo_c[:, 0:CW],
                                                   mybir.AluOpType.mult,
                                                   mybir.AluOpType.add)
                    nc.vector.tensor_reduce(mx_sb[:, cidx:cidx + 1], wslice,
                                            mybir.AxisListType.X,
                                            mybir.AluOpType.max)
                    nmx_c = sm.tile([1, 1], F32, tag="nmxc", bufs=2)
                    nc.vector.tensor_scalar_mul(nmx_c[:], mx_sb[:, cidx:cidx + 1],
                                                -1.0)
                    esc = sm.tile([1, 512], F16, tag="esc", bufs=2)
                    nc.scalar.activation(esc[:, 0:CW], wslice,
                                         mybir.ActivationFunctionType.Exp,
                                         bias=nmx_c[:],
                                         accum_out=sm_sb[:, cidx:cidx + 1])

            # ---------- local stats -> global logsumexp ----------
            m_loc = sm.tile([1, 1], F32, tag="mloc")
            nc.vector.tensor_reduce(m_loc[:], mx_sb[:], mybir.AxisListType.X,
                                    mybir.AluOpType.max)
            dm = sm.tile([1, n_chunks], F32, tag="dm")
            nc.vector.tensor_scalar(dm[:], mx_sb[:], m_loc[:], None,
                                    mybir.AluOpType.subtract)
            edm = sm.tile([1, n_chunks], F32, tag="edm")
            nc.scalar.activation(edm[:], dm[:], mybir.ActivationFunctionType.Exp)
            sadj = sm.tile([1, n_chunks], F32, tag="sadj")
            nc.vector.tensor_mul(sadj[:], sm_sb[:], edm[:])
            s_loc = sm.tile([1, 1], F32, tag="sloc")
            nc.vector.tensor_reduce(s_loc[:], sadj[:], mybir.AxisListType.X,
                                    mybir.AluOpType.add)
            pack = sm.tile([1, 8], F32, tag="pack")
            nc.vector.tensor_copy(pack[:, 0:1], m_loc[:])
            nc.vector.tensor_copy(pack[:, 1:2], s_loc[:])
            ag_s_in = dr.tile([1, 8], F32, tag="agsi")
            ag_s_out = dr.tile([NC, 8], F32, tag="agso")
            nc.gpsimd.dma_start(ag_s_in[:], pack[:])
            nc.gpsimd.collective_compute(
                "AllGather", mybir.AluOpType.bypass,
                ins=[ag_s_in.opt()], outs=[ag_s_out.opt()], replica_groups=rg)
            stat = sm.tile([1, NC, 8], F32, tag="stat")
            nc.gpsimd.dma_start(stat[:], ag_s_out[:].rearrange("r n -> () r n"))
            m_all = stat[:, :, 0]
            s_all = stat[:, :, 1]
            gm = sm.tile([1, 1], F32, tag="gm")
            nc.vector.tensor_reduce(gm[:], m_all, mybir.AxisListType.X,
                                    mybir.AluOpType.max)
            dg = sm.tile([1, NC], F32, tag="dg")
            nc.vector.tensor_scalar(dg[:], m_all, gm[:], None,
                                    mybir.AluOpType.subtract)
            edg = sm.tile([1, NC], F32, tag="edg")
            nc.scalar.activation(edg[:], dg[:], mybir.ActivationFunctionType.Exp)
            sg2 = sm.tile([1, NC], F32, tag="sg2")
            nc.vector.tensor_mul(sg2[:], s_all, edg[:])
            ssum = sm.tile([1, 1], F32, tag="ssum")
            nc.vector.tensor_reduce(ssum[:], sg2[:], mybir.AxisListType.X,
                                    mybir.AluOpType.add)
            lns2 = sm.tile([1, 1], F32, tag="lns2")
            nc.scalar.activation(lns2[:], ssum[:], mybir.ActivationFunctionType.Ln)
            lse = sm.tile([1, 1], F32, tag="lse")
            nc.vector.tensor_add(lse[:], gm[:], lns2[:])
            nlse = sm.tile([1, 1], F32, tag="nlse")
            nc.vector.tensor_scalar_mul(nlse[:], lse[:], -1.0)

            # ---------- out = word - lse (in SBUF, vector+scalar halves) ----
            QT = SV // 8
            for hf in range(8):
                ob = sm.tile([1, QT], F32, tag="ob", bufs=2)
                src = word_sb[:, hf * QT:(hf + 1) * QT]
                if hf % 2 == 0:
                    nc.vector.tensor_scalar(ob[:], src, lse[:], None,
                                            mybir.AluOpType.subtract)
                else:
                    nc.scalar.activation(ob[:], src,
                                         mybir.ActivationFunctionType.Identity,
                                         bias=nlse[:])
                nc.gpsimd.dma_start(d_out[:, hf * QT:(hf + 1) * QT], ob[:])

    nc.compile()
    return nc


def _get_nc():
    if 'nc' not in _compiled:
        _compiled['nc'] = _build()
    return _compiled['nc']


def _shard_inputs(encoder_outputs, h0, c0, x0, W_attn, b_attn, W_comb, b_comb,
                  W_ih, b_ih, W_hh, b_hh, W_out, b_out):
    f = lambda a: np.ascontiguousarray(np.asarray(a), dtype=np.float32)
    E = f(encoder_outputs); W_attn = f(W_attn); W_comb = f(W_comb)
    W_ih = f(W_ih); W_hh = f(W_hh); W_out = f(W_out)
    h0f = f(h0).reshape(H)
    x0f = f(x0).reshape(H).astype(np.float16)
    c0f = f(c0).reshape(H)
    b_attn = f(b_attn); b_comb = f(b_comb); b_out = f(b_out)
    bg_full = f(b_ih) + f(b_hh)

    h0_16 = h0f.astype(np.float16)
    # h08[p, s, d] = h0[32p + 16s + d]
    h08 = np.ascontiguousarray(h0f.reshape(128, 2, ND)).astype(E4)

    # E chunks: [blk, p, j, n] = E[32p + 8blk + j, h0+n]
    E_r = E.reshape(128, 32, H)

    in_maps = []
    for k in range(NC):
        l0, hh0, v0 = k * SL, k * SH, k * SV
        wa = W_attn[l0:l0 + SL].T.reshape(128, 8, 8, SL) \
            .transpose(1, 0, 2, 3).reshape(8, 128, 8 * SL)
        e = E_r[:, :, hh0:hh0 + SH].reshape(128, 4, 8, SH) \
            .transpose(1, 0, 2, 3).reshape(4, 128, 8 * SH)
        wc = W_comb[hh0:hh0 + SH].T.reshape(128, 8, 8, SH) \
            .transpose(1, 0, 2, 3).reshape(8, 128, 8 * SH)
        rows = np.concatenate([np.arange(g * H + hh0, g * H + hh0 + SH)
                               for g in range(4)])
        wih = W_ih[rows].T.reshape(128, 16, 2, 2048) \
            .transpose(1, 0, 2, 3).reshape(16, 128, 4096)
        # whh8[d, p, s, c] = W_hh[rows[c], 32p + 16s + d], natural scale fp8
        whh8 = np.ascontiguousarray(
            W_hh[rows].T.reshape(128, 2, ND, 4 * SH).transpose(2, 0, 1, 3)
        ).astype(E4)
        # wo8[g, d, p, s, n] = 128 * W_out[v0 + g*2000 + n, 32p + 16s + d]
        Ws = W_out[v0:v0 + SV].T * WSC
        wo8 = np.ascontiguousarray(
            Ws.reshape(128, 2, ND, NG, NCH * CW).transpose(3, 2, 0, 1, 4)
        ).astype(E4)
        in_maps.append({
            "h0f": h0_16, "x0f": x0f, "h08": h08,
            "c0s": np.ascontiguousarray(c0f[hh0:hh0 + SH]),
            "ba": np.ascontiguousarray(b_attn[l0:l0 + SL]),
            "bc": np.ascontiguousarray(b_comb[hh0:hh0 + SH]),
            "bg": np.ascontiguousarray(bg_full[rows]),
            "bo": np.ascontiguousarray(b_out[v0:v0 + SV]),
            "wa": np.ascontiguousarray(wa, np.float16),
            "e": np.ascontiguousarray(e, np.float16),
            "wc": np.ascontiguousarray(wc, np.float16),
            "wih": np.ascontiguousarray(wih, np.float16),
            "whh": whh8,
            "wo": wo8,
        })
    return in_maps


def _run(in_maps, trace=False):
    nc = _get_nc()
    return run_bass_kernel_spmd(nc, in_maps, list(range(NC)), trace=trace)


def kernel(**inputs):
    in_maps = _shard_inputs(**inputs)
    res = _run(in_maps)
    return np.concatenate([res.results[k]["out"] for k in range(NC)], axis=1)


def run_traced(**inputs):
    """test-only helper: returns (output, BassKernelResults with profiling)."""
    in_maps = _shard_inputs(**inputs)
    res = _run(in_maps, trace=True)
    out = np.concatenate([res.results[k]["out"] for k in range(NC)], axis=1)
    return out, res


# revision 6
# speedup vs baseline: 1.5459x; 1.1425x over previous
"""Bass/Trainium2 kernel for nn_DecoderAttn: batch-1 attention decoder step.

Sharding over 8 NeuronCores (tensor-parallel):
  - W_attn row-split (L dim): each core computes 512 attn logits -> AllGather
  - encoder_outputs col-split (H dim): each core computes 512 of attn_applied -> AllGather
  - W_comb row-split: 512 of lstm_in -> AllGather
  - W_ih/W_hh row-split by hidden slice (512 rows of each gate): LSTM math on
    a 512-slice of (c, h) -> AllGather h_new
  - W_out row-split (vocab dim): 16000 logits/core; log_softmax via an
    AllGather of per-core (max, sumexp) stats.

Precision plan (validated vs fp32 reference host-side, rel err ~6e-3):
  - W_out: fp8 e4m3 scaled x128 + DoubleRow matmul (2 k-subtiles of 128 per
    instruction) -> half the HBM bytes at full column rate; psum carries
    128*logits, rescaled in the fused bias-add.
  - W_hh: fp8 e4m3 natural scale + DoubleRow; its term is ~0.1% of the gate
    magnitude so fp8 noise there is negligible; accumulates into the same
    psum group as the fp16 W_ih matmuls.
  - Everything else fp16 weights / fp32 psum (fp8 anywhere else flips LSTM
    gates: measured 3-5e-2 rel err host-side).

DMA queues: chain weights (W_attn/E/W_comb/W_hh/W_ih) stream on the sync
ring; W_out streams on the scalar ring (all 128 tile DMAs issued up-front,
paced by the wop pool ring's WAR semaphores); small loads ride vector;
gpsimd handles collective staging. Stage-E logits stay in SBUF (fp16) and
the final log_softmax subtraction happens in-SBUF (vector+scalar halves),
removing the baseline's serialized DRAM round-trip tail.
"""

import sys

if '/opt/trn_rl_repo' not in sys.path:
    sys.path.insert(0, '/opt/trn_rl_repo')

import numpy as np
import ml_dtypes

import concourse.bass as bass
import concourse.bacc as bacc
import concourse.tile as tile
import concourse.mybir as mybir
from concourse.bass_utils import run_bass_kernel_spmd

F32 = mybir.dt.float32
F16 = mybir.dt.float16
F8 = mybir.dt.float8e4
DR = mybir.MatmulPerfMode.DoubleRow
E4 = ml_dtypes.float8_e4m3

H = 4096
L = 4096
V = 128000
NC = 8
SH = H // NC        # 512 hidden slice
SL = L // NC        # 512 logit slice
SV = V // NC        # 16000 vocab slice
ND = H // 256       # 16 DoubleRow k-pairs
NG = 8              # stage-E groups (4 chunks of 500 each)
NCH = 4             # chunks per group
CW = 500            # chunk width (NG*NCH*CW == SV)
WSC = 128.0         # W_out fp8 scale

N_I = H // 128      # 32 contraction chunks for K=4096
N_I2 = 2 * H // 128  # 64 for K=8192

_compiled = {}


def _build():
    nc = bacc.Bacc("TRN2", target_bir_lowering=False, debug=False, num_devices=NC)

    # ---- kernel I/O (per-core shards, same names across cores) ----
    d_h0 = nc.dram_tensor("h0f", [H], F16, kind="ExternalInput")
    d_x0 = nc.dram_tensor("x0f", [H], F16, kind="ExternalInput")
    d_h08 = nc.dram_tensor("h08", [128, 2, ND], F8, kind="ExternalInput")
    d_c0 = nc.dram_tensor("c0s", [SH], F32, kind="ExternalInput")
    d_ba = nc.dram_tensor("ba", [SL], F32, kind="ExternalInput")
    d_bc = nc.dram_tensor("bc", [SH], F32, kind="ExternalInput")
    d_bg = nc.dram_tensor("bg", [4 * SH], F32, kind="ExternalInput")
    d_bo = nc.dram_tensor("bo", [SV], F32, kind="ExternalInput")
    d_wa = nc.dram_tensor("wa", [8, 128, 8 * SL], F16, kind="ExternalInput")
    d_e = nc.dram_tensor("e", [4, 128, 8 * SH], F16, kind="ExternalInput")
    d_wc = nc.dram_tensor("wc", [8, 128, 8 * SH], F16, kind="ExternalInput")
    d_whh = nc.dram_tensor("whh", [ND, 128, 2, 4 * SH], F8, kind="ExternalInput")
    d_wih = nc.dram_tensor("wih", [16, 128, 2 * 2048], F16, kind="ExternalInput")
    d_wo = nc.dram_tensor("wo", [NG, ND, 128, 2, NCH * CW], F8, kind="ExternalInput")
    d_out = nc.dram_tensor("out", [1, SV], F32, kind="ExternalOutput")

    rg = [list(range(NC))]

    with tile.TileContext(nc) as tc:
        with (
            tc.tile_pool(name="singles", bufs=1) as sg,
            tc.tile_pool(name="cw", bufs=6) as cw,        # chain weight stream
            tc.tile_pool(name="wop", bufs=14) as wop,     # W_out stream
            tc.tile_pool(name="small", bufs=1) as sm,     # small working tiles
            tc.tile_pool(name="psum", bufs=1, space="PSUM") as ps,
            tc.tile_pool(name="dram", bufs=1, space="DRAM") as dr,
        ):
            # ---------- rank alignment barrier ----------
            bar_in = dr.tile([1, 8], F32, tag="bar_in")
            bar_out = dr.tile([NC, 8], F32, tag="bar_out")
            zt = sg.tile([1, 8], F32, tag="zt")
            nc.gpsimd.memset(zt[:], 0.0)
            nc.gpsimd.dma_start(bar_in[:], zt[:])
            nc.gpsimd.collective_compute(
                "AllGather", mybir.AluOpType.bypass,
                ins=[bar_in.opt()], outs=[bar_out.opt()], replica_groups=rg)

            # ---------- small loads (sync ring, ahead of the chain stream) --
            hx = sg.tile([128, 64], F16, tag="hx")       # [h; x], elem 64p+i
            nc.sync.dma_start(hx[0:64, :], d_h0[:].rearrange("(p i) -> p i", p=64))
            nc.sync.dma_start(hx[64:128, :], d_x0[:].rearrange("(p i) -> p i", p=64))
            h08 = sg.tile([128, 2, ND], F8, tag="h08")   # h for W_hh DoubleRow
            nc.sync.dma_start(h08[:], d_h08[:])
            xc = sg.tile([128, 64], F16, tag="xc")       # stage-C lhsT (x half)
            nc.sync.dma_start(xc[0:64, :], d_x0[:].rearrange("(p i) -> p i", p=64))
            c0t = sg.tile([1, SH], F32, tag="c0t")
            nc.sync.dma_start(c0t[:], d_c0[:].rearrange("n -> () n"))
            bat = sg.tile([1, SL], F32, tag="bat")
            nc.sync.dma_start(bat[:], d_ba[:].rearrange("n -> () n"))
            bct = sg.tile([1, SH], F32, tag="bct")
            nc.sync.dma_start(bct[:], d_bc[:].rearrange("n -> () n"))
            bgt = sg.tile([1, 4 * SH], F32, tag="bgt")
            nc.sync.dma_start(bgt[:], d_bg[:].rearrange("n -> () n"))

            # ---------- chain weight stream: issue ALL dmas up-front ----
            # (sync ring; paced by the cw pool's 6-slot WAR ring)
            wa_t, e_t, wc_t, whh_t, wih_t = [], [], [], [], []
            for b in range(8):
                t = cw.tile([128, 8, SL], F16, tag="cw", name=f"wa{b}")
                nc.sync.dma_start(t[:], d_wa[b].rearrange("p (j n) -> p j n", j=8))
                wa_t.append(t)
            for b in range(4):
                t = cw.tile([128, 8, SH], F16, tag="cw", name=f"e{b}")
                nc.sync.dma_start(t[:], d_e[b].rearrange("p (j n) -> p j n", j=8))
                e_t.append(t)
            for b in range(ND):
                t = cw.tile([128, 2, 4 * SH], F8, tag="cw", name=f"whh{b}")
                nc.sync.dma_start(t[:], d_whh[b])
                whh_t.append(t)
            for b in range(8):
                t = cw.tile([128, 8, SH], F16, tag="cw", name=f"wc{b}")
                nc.sync.dma_start(t[:], d_wc[b].rearrange("p (j n) -> p j n", j=8))
                wc_t.append(t)
            for b in range(16):
                t = cw.tile([128, 2, 2048], F16, tag="cw", name=f"wih{b}")
                nc.sync.dma_start(t[:], d_wih[b].rearrange("p (j n) -> p j n", j=2))
                wih_t.append(t)

            # ---------- W_out stream ----------
            # First 14 tiles (the wop ring's prefetch window) issue on the
            # scalar ring immediately; the rest ride the sync ring (idle
            # after the chain stream) so scalar's activations aren't stuck
            # behind WAR-paced dma issues for the whole of stage E.
            wo_t = []
            for k in range(NG * ND):
                g, d = divmod(k, ND)
                t = wop.tile([128, 2, NCH * CW], F8, tag="wo",
                             name=f"wo{g}_{d}")
                eng = nc.scalar if k < 14 else nc.sync
                eng.dma_start(t[:], d_wo[g, d])
                wo_t.append(t)


            # ---------- stage A: attn logits = [h;x] @ W_attn^T ----------
            pa = ps.tile([1, SL], F32, tag="po", bufs=4)
            for b in range(8):
                for j in range(8):
                    i = 8 * b + j
                    nc.tensor.matmul(pa[:], hx[:, i:i + 1], wa_t[b][:, j, :],
                                     start=(i == 0), stop=(i == N_I2 - 1))
            logits_loc = sm.tile([1, SL], F16, tag="vloc", bufs=2)
            nc.vector.tensor_add(logits_loc[:], pa[:], bat[:])
            ag_a_in = dr.tile([1, SL], F16, tag="agai")
            ag_a_out = dr.tile([NC, SL], F16, tag="agao")
            nc.gpsimd.dma_start(ag_a_in[:], logits_loc[:])
            nc.gpsimd.collective_compute(
                "AllGather", mybir.AluOpType.bypass,
                ins=[ag_a_in.opt()], outs=[ag_a_out.opt()], replica_groups=rg)

            # ---------- stage B: attn_applied with folded log_softmax ----------
            # aa = log_softmax(l) @ E = l @ E - logZ * (1^T E)
            aw = sg.tile([128, 32], F16, tag="aw")       # raw logits
            nc.gpsimd.dma_start(
                aw[:],
                ag_a_out[:].rearrange("r n -> (r n)").rearrange("(p i) -> p i", p=128))
            lfl = sm.tile([1, L], F16, tag="lfl")        # logits, free-major
            nc.gpsimd.dma_start(
                lfl[:], ag_a_out[:].rearrange("r n -> (r n)").rearrange("n -> () n"))
            mxb = sm.tile([1, 1], F32, tag="mxb")
            nc.vector.tensor_reduce(mxb[:], lfl[:], mybir.AxisListType.X,
                                    mybir.AluOpType.max)
            nmxb = sm.tile([1, 1], F32, tag="nmxb")
            nc.vector.tensor_scalar_mul(nmxb[:], mxb[:], -1.0)
            s1 = sm.tile([1, 1], F32, tag="s1")
            nc.scalar.activation(lfl[:], lfl[:], mybir.ActivationFunctionType.Exp,
                                 bias=nmxb[:], accum_out=s1[:])
            lnsb = sm.tile([1, 1], F32, tag="lnsb")
            nc.scalar.activation(lnsb[:], s1[:], mybir.ActivationFunctionType.Ln)
            lzb = sm.tile([1, 1], F32, tag="lzb")
            nc.vector.tensor_add(lzb[:], mxb[:], lnsb[:])
            nlz = sm.tile([1, 1], F32, tag="nlz")
            nc.vector.tensor_scalar_mul(nlz[:], lzb[:], -1.0)
            ones = sg.tile([128, 1], F16, tag="ones")
            nc.vector.memset(ones[:], 1.0)
            pb = ps.tile([1, SH], F32, tag="po", bufs=4)
            pcs = ps.tile([1, SH], F32, tag="po", bufs=4, name="pcs")
            # colsum = 1^T E needs no AG result: fills the AllGather stall
            for b in range(4):
                for j in range(8):
                    i = 8 * b + j
                    nc.tensor.matmul(pcs[:], ones[:], e_t[b][:, j, :],
                                     start=(i == 0), stop=(i == N_I - 1))
            # h @ W_hh^T (fp8 DoubleRow): needs only h0 -- consumes the
            # chain stream during the dispatch-skew / AG_a window
            pg = ps.tile([1, 4 * SH], F32, tag="pg")
            for d in range(ND):
                for b in range(4):
                    nc.tensor.matmul(pg[:, 512 * b:512 * (b + 1)],
                                     h08[:, :, d:d + 1],
                                     whh_t[d][:, :, 512 * b:512 * (b + 1)],
                                     start=(d == 0), stop=False, perf_mode=DR)
            for b in range(4):
                for j in range(8):
                    i = 8 * b + j
                    nc.tensor.matmul(pb[:], aw[:, i:i + 1], e_t[b][:, j, :],
                                     start=(i == 0), stop=(i == N_I - 1))
            cs_sb = sm.tile([1, SH], F32, tag="cs_sb")
            nc.vector.tensor_copy(cs_sb[:], pcs[:])
            aa_loc = sm.tile([1, SH], F16, tag="vloc", bufs=2)
            nc.vector.scalar_tensor_tensor(aa_loc[:], cs_sb[:], nlz[:], pb[:],
                                           mybir.AluOpType.mult,
                                           mybir.AluOpType.add)
            ag_b_in = dr.tile([1, SH], F16, tag="agbi")
            ag_b_out = dr.tile([NC, SH], F16, tag="agbo")
            nc.gpsimd.dma_start(ag_b_in[:], aa_loc[:])
            nc.gpsimd.collective_compute(
                "AllGather", mybir.AluOpType.bypass,
                ins=[ag_b_in.opt()], outs=[ag_b_out.opt()], replica_groups=rg)

            # ---------- stage C: lstm_in = [x; aa] @ W_comb^T ----------
            nc.gpsimd.dma_start(
                xc[64:128, :],
                ag_b_out[:].rearrange("r n -> (r n)").rearrange("(p i) -> p i", p=64))
            pc = ps.tile([1, SH], F32, tag="po", bufs=4)
            for b in range(8):
                for j in range(8):
                    i = 8 * b + j
                    nc.tensor.matmul(pc[:], xc[:, i:i + 1], wc_t[b][:, j, :],
                                     start=(i == 0), stop=(i == N_I2 - 1))
            li_loc = sm.tile([1, SH], F16, tag="vloc", bufs=2)
            nc.vector.tensor_add(li_loc[:], pc[:], bct[:])
            ag_c_in = dr.tile([1, SH], F16, tag="agci")
            ag_c_out = dr.tile([NC, SH], F16, tag="agco")
            nc.gpsimd.dma_start(ag_c_in[:], li_loc[:])
            nc.gpsimd.collective_compute(
                "AllGather", mybir.AluOpType.bypass,
                ins=[ag_c_in.opt()], outs=[ag_c_out.opt()], replica_groups=rg)

            # ---------- stage D: gates + LSTM cell ----------
            li = sg.tile([128, 32], F16, tag="li")
            nc.gpsimd.dma_start(
                li[:],
                ag_c_out[:].rearrange("r n -> (r n)").rearrange("(p i) -> p i", p=128))
            # + lstm_in @ W_ih^T (fp16; pg already carries h @ W_hh^T)
            for blk in range(16):
                for j in range(2):
                    i = 2 * blk + j
                    for b in range(4):
                        nc.tensor.matmul(pg[:, 512 * b:512 * (b + 1)],
                                         li[:, i:i + 1],
                                         wih_t[blk][:, j, 512 * b:512 * (b + 1)],
                                         start=False, stop=(i == N_I - 1))
            gsb = sm.tile([1, 4 * SH], F32, tag="gsb")
            nc.vector.tensor_add(gsb[:], pg[:], bgt[:])
            s_i = sm.tile([1, SH], F32, tag="si")
            s_f = sm.tile([1, SH], F32, tag="sf")
            t_g = sm.tile([1, SH], F32, tag="tg")
            s_o = sm.tile([1, SH], F32, tag="so")
            Sg = mybir.ActivationFunctionType.Sigmoid
            Th = mybir.ActivationFunctionType.Tanh
            nc.scalar.activation(s_i[:], gsb[:, 0:SH], Sg)
            nc.scalar.activation(s_f[:], gsb[:, SH:2 * SH], Sg)
            nc.scalar.activation(s_o[:], gsb[:, 3 * SH:4 * SH], Sg)
            nc.scalar.activation(t_g[:], gsb[:, 2 * SH:3 * SH], Th)
            nc.vector.tensor_mul(s_f[:], s_f[:], c0t[:])
            nc.vector.tensor_mul(s_i[:], s_i[:], t_g[:])
            nc.vector.tensor_add(s_f[:], s_f[:], s_i[:])
            tc_n = sm.tile([1, SH], F32, tag="tcn")
            nc.scalar.activation(tc_n[:], s_f[:], Th)
            hn_loc = sm.tile([1, SH], F16, tag="vloc", bufs=2)
            nc.vector.tensor_mul(hn_loc[:], s_o[:], tc_n[:])
            ag_h_in = dr.tile([1, SH], F16, tag="aghi")
            ag_h_out = dr.tile([NC, SH], F16, tag="agho")
            nc.gpsimd.dma_start(ag_h_in[:], hn_loc[:])
            nc.gpsimd.collective_compute(
                "AllGather", mybir.AluOpType.bypass,
                ins=[ag_h_in.opt()], outs=[ag_h_out.opt()], replica_groups=rg)

            # ---------- stage E: word = h_new @ W_out^T (fp8 DoubleRow) ----
            hn16 = sg.tile([128, 2, ND], F16, tag="hn16")
            nc.gpsimd.dma_start(
                hn16[:],
                ag_h_out[:].rearrange("r n -> (r n)")
                .rearrange("(p s d) -> p s d", p=128, s=2))
            hn8 = sg.tile([128, 2, ND], F8, tag="hn8")
            nc.vector.tensor_copy(hn8[:], hn16[:])

            word_sb = sg.tile([1, SV], F16, tag="word")
            n_chunks = NG * NCH
            mx_sb = sg.tile([1, n_chunks], F32, tag="mxsb")
            sm_sb = sg.tile([1, n_chunks], F32, tag="smsb")
            for g in range(NG):
                po = [ps.tile([1, 512], F32, tag="po", bufs=4, name=f"po{g}_{c}")
                      for c in range(NCH)]
                for d in range(ND):
                    wt = wo_t[g * ND + d]
                    for c in range(NCH):
                        nc.tensor.matmul(po[c][:, 0:CW], hn8[:, :, d:d + 1],
                                         wt[:, :, CW * c:CW * (c + 1)],
                                         start=(d == 0), stop=(d == ND - 1),
                                         perf_mode=DR)
                for c in range(NCH):
                    cidx = NCH * g + c
                    vabs = cidx * CW
                    bo_c = sm.tile([1, 512], F32, tag="boc", bufs=3)
                    nc.gpsimd.dma_start(bo_c[:, 0:CW],
                                        d_bo[vabs:vabs + CW].rearrange("n -> () n"))
                    # word = psum/128 + b_out (fused rescale of the x128 fp8)
                    wslice = word_sb[:, vabs:vabs + CW]
                    nc.vector.scalar_tensor_tensor(wslice, po[c][:, 0:CW],
                                                   1.0 / WSC, bo_c[:, 0:CW],
                                                   mybir.AluOpType.mult,
                                                   mybir.AluOpType.add)
                    nc.vector.tensor_reduce(mx_sb[:, cidx:cidx + 1], wslice,
                                            mybir.AxisListType.X,
                                            mybir.AluOpType.max)
                    nmx_c = sm.tile([1, 1], F32, tag="nmxc", bufs=2)
                    nc.vector.tensor_scalar_mul(nmx_c[:], mx_sb[:, cidx:cidx + 1],
                                                -1.0)
                    esc = sm.tile([1, 512], F16, tag="esc", bufs=2)
                    nc.scalar.activation(esc[:, 0:CW], wslice,
                                         mybir.ActivationFunctionType.Exp,
                                         bias=nmx_c[:],
                                         accum_out=sm_sb[:, cidx:cidx + 1])

            # ---------- local stats -> global logsumexp ----------
            m_loc = sm.tile([1, 1], F32, tag="mloc")
            nc.vector.tensor_reduce(m_loc[:], mx_sb[:], mybir.AxisListType.X,
                                    mybir.AluOpType.max)
            dm = sm.tile([1, n_chunks], F32, tag="dm")
            nc.vector.tensor_scalar(dm[:], mx_sb[:], m_loc[:], None,
                                    mybir.AluOpType.subtract)
            edm = sm.tile([1, n_chunks], F32, tag="edm")
            nc.scalar.activation(edm[:], dm[:], mybir.ActivationFunctionType.Exp)
            sadj = sm.tile([1, n_chunks], F32, tag="sadj")
            nc.vector.tensor_mul(sadj[:], sm_sb[:], edm[:])
            s_loc = sm.tile([1, 1], F32, tag="sloc")
            nc.vector.tensor_reduce(s_loc[:], sadj[:], mybir.AxisListType.X,
                                    mybir.AluOpType.add)
            pack = sm.tile([1, 8], F32, tag="pack")
            nc.vector.tensor_copy(pack[:, 0:1], m_loc[:])
            nc.vector.tensor_copy(pack[:, 1:2], s_loc[:])
            ag_s_in = dr.tile([1, 8], F32, tag="agsi")
            ag_s_out = dr.tile([NC, 8], F32, tag="agso")
            nc.gpsimd.dma_start(ag_s_in[:], pack[:])
            nc.gpsimd.collective_compute(
                "AllGather", mybir.AluOpType.bypass,
                ins=[ag_s_in.opt()], outs=[ag_s_out.opt()], replica_groups=rg)
            stat = sm.tile([1, NC, 8], F32, tag="stat")
            nc.gpsimd.dma_start(stat[:], ag_s_out[:].rearrange("r n -> () r n"))
            m_all = stat[:, :, 0]
            s_all = stat[:, :, 1]
            gm = sm.tile([1, 1], F32, tag="gm")
            nc.vector.tensor_reduce(gm[:], m_all, mybir.AxisListType.X,
                                    mybir.AluOpType.max)
            dg = sm.tile([1, NC], F32, tag="dg")
            nc.vector.tensor_scalar(dg[:], m_all, gm[:], None,
                                    mybir.AluOpType.subtract)
            edg = sm.tile([1, NC], F32, tag="edg")
            nc.scalar.activation(edg[:], dg[:], mybir.ActivationFunctionType.Exp)
            sg2 = sm.tile([1, NC], F32, tag="sg2")
            nc.vector.tensor_mul(sg2[:], s_all, edg[:])
            ssum = sm.tile([1, 1], F32, tag="ssum")
            nc.vector.tensor_reduce(ssum[:], sg2[:], mybir.AxisListType.X,
                                    mybir.AluOpType.add)
            lns2 = sm.tile([1, 1], F32, tag="lns2")
            nc.scalar.activation(lns2[:], ssum[:], mybir.ActivationFunctionType.Ln)
            lse = sm.tile([1, 1], F32, tag="lse")
            nc.vector.tensor_add(lse[:], gm[:], lns2[:])
            nlse = sm.tile([1, 1], F32, tag="nlse")
            nc.vector.tensor_scalar_mul(nlse[:], lse[:], -1.0)

            # ---------- out = word - lse (in SBUF, vector+scalar halves) ----
            QT = SV // 8
            for hf in range(8):
                ob = sm.tile([1, QT], F32, tag="ob", bufs=2)
                src = word_sb[:, hf * QT:(hf + 1) * QT]
                if hf % 2 == 0:
                    nc.vector.tensor_scalar(ob[:], src, lse[:], None,
                                            mybir.AluOpType.subtract)
                else:
                    nc.scalar.activation(ob[:], src,
                                         mybir.ActivationFunctionType.Identity,
                                         bias=nlse[:])
                nc.gpsimd.dma_start(d_out[:, hf * QT:(hf + 1) * QT], ob[:])

    nc.compile()
    return nc


def _get_nc():
    if 'nc' not in _compiled:
        _compiled['nc'] = _build()
    return _compiled['nc']


def _shard_inputs(encoder_outputs, h0, c0, x0, W_attn, b_attn, W_comb, b_comb,
                  W_ih, b_ih, W_hh, b_hh, W_out, b_out):
    f = lambda a: np.ascontiguousarray(np.asarray(a), dtype=np.float32)
    E = f(encoder_outputs); W_attn = f(W_attn); W_comb = f(W_comb)
    W_ih = f(W_ih); W_hh = f(W_hh); W_out = f(W_out)
    h0f = f(h0).reshape(H)
    x0f = f(x0).reshape(H).astype(np.float16)
    c0f = f(c0).reshape(H)
    b_attn = f(b_attn); b_comb = f(b_comb); b_out = f(b_out)
    bg_full = f(b_ih) + f(b_hh)

    h0_16 = h0f.astype(np.float16)
    # h08[p, s, d] = h0[32p + 16s + d]
    h08 = np.ascontiguousarray(h0f.reshape(128, 2, ND)).astype(E4)

    # E chunks: [blk, p, j, n] = E[32p + 8blk + j, h0+n]
    E_r = E.reshape(128, 32, H)

    in_maps = []
    for k in range(NC):
        l0, hh0, v0 = k * SL, k * SH, k * SV
        wa = W_attn[l0:l0 + SL].T.reshape(128, 8, 8, SL) \
            .transpose(1, 0, 2, 3).reshape(8, 128, 8 * SL)
        e = E_r[:, :, hh0:hh0 + SH].reshape(128, 4, 8, SH) \
            .transpose(1, 0, 2, 3).reshape(4, 128, 8 * SH)
        wc = W_comb[hh0:hh0 + SH].T.reshape(128, 8, 8, SH) \
            .transpose(1, 0, 2, 3).reshape(8, 128, 8 * SH)
        rows = np.concatenate([np.arange(g * H + hh0, g * H + hh0 + SH)
                               for g in range(4)])
        wih = W_ih[rows].T.reshape(128, 16, 2, 2048) \
            .transpose(1, 0, 2, 3).reshape(16, 128, 4096)
        # whh8[d, p, s, c] = W_hh[rows[c], 32p + 16s + d], natural scale fp8
        whh8 = np.ascontiguousarray(
            W_hh[rows].T.reshape(128, 2, ND, 4 * SH).transpose(2, 0, 1, 3)
        ).astype(E4)
        # wo8[g, d, p, s, n] = 128 * W_out[v0 + g*2000 + n, 32p + 16s + d]
        Ws = W_out[v0:v0 + SV].T * WSC
        wo8 = np.ascontiguousarray(
            Ws.reshape(128, 2, ND, NG, NCH * CW).transpose(3, 2, 0, 1, 4)
        ).astype(E4)
        in_maps.append({
            "h0f": h0_16, "x0f": x0f, "h08": h08,
            "c0s": np.ascontiguousarray(c0f[hh0:hh0 + SH]),
            "ba": np.ascontiguousarray(b_attn[l0:l0 + SL]),
            "bc": np.ascontiguousarray(b_comb[hh0:hh0 + SH]),
            "bg": np.ascontiguousarray(bg_full[rows]),
            "bo": np.ascontiguousarray(b_out[v0:v0 + SV]),
            "wa": np.ascontiguousarray(wa, np.float16),
            "e": np.ascontiguousarray(e, np.float16),
            "wc": np.ascontiguousarray(wc, np.float16),
            "wih": np.ascontiguousarray(wih, np.float16),
            "whh": whh8,
            "wo": wo8,
        })
    return in_maps


def _run(in_maps, trace=False):
    nc = _get_nc()
    return run_bass_kernel_spmd(nc, in_maps, list(range(NC)), trace=trace)


def kernel(**inputs):
    in_maps = _shard_inputs(**inputs)
    res = _run(in_maps)
    return np.concatenate([res.results[k]["out"] for k in range(NC)], axis=1)


def run_traced(**inputs):
    """test-only helper: returns (output, BassKernelResults with profiling)."""
    in_maps = _shard_inputs(**inputs)
    res = _run(in_maps, trace=True)
    out = np.concatenate([res.results[k]["out"] for k in range(NC)], axis=1)
    return out, res
